# revision 13
# baseline (speedup 1.0000x reference)
"""Trainium2 Bass kernel for the BoundedMemory module.

Contract: kernel(**inputs) takes FULL unsharded numpy inputs (as produced by
the reference setup_inputs()) and returns the FULL output tuple:
  (context, all_scores, ek, ev, es, ep, sk, sv, ss, sp)

Sharding: pure data-parallel over batch (B=32 -> 4 per core x 8 cores).

Per-core device program (all shapes hardcoded):
 - keys banks stream through SBUF in whole-bank 4MiB tiles (partition p holds
   64 contiguous slot rows); DVE tensor_tensor_reduce computes q.k dot
   products; the same SBUF tile is DMA'd back out as the updated-bank copy.
 - values banks + score banks are copied DRAM->DRAM (never enter SBUF).
 - exact top-32 via hierarchical vector.max/match_replace reduction, then
   max_index over row-layout scores for the slot indices; top-32 value rows
   are fetched with an indirect gather DMA.
 - circular-buffer scatter writes are done with indirect DMAs whose row
   offsets are computed on device (masked rows redirect to padding rows).
"""

import numpy as np

import concourse.bacc as bacc
import concourse.bass as bass
import concourse.mybir as mybir
import concourse.tile as tile
from concourse.bass_utils import run_bass_kernel_spmd
from concourse.masks import make_identity

B = 32
N_CORES = 8
BL = B // N_CORES  # batch per core = 4
CHUNK = 128
DM = 1024  # d_model
KD = 128  # key_dim == val_dim
S = 8192  # slots per bank
TOPK = 32
ROWS = BL * S  # 32768 rows per bank tensor per core
PAD = 8  # padding rows for masked-write redirect
F32 = mybir.dt.float32
I32 = mybir.dt.int32
U32 = mybir.dt.uint32
NEG = -1.0e30
DEBUG_TAPS = False


def _build_program():
    nc = bacc.Bacc("TRN2", target_bir_lowering=False, debug=False)

    def din(name, shape, dt=F32):
        return nc.dram_tensor(name, shape, dt, kind="ExternalInput").ap()

    def dout(name, shape, dt=F32):
        return nc.dram_tensor(name, shape, dt, kind="ExternalOutput").ap()

    io = {}
    io["hidden"] = din("hidden", [BL, CHUNK, DM])
    io["query"] = din("query", [BL, DM])
    io["ws_epi"] = din("ws_epi", [BL, 1])
    io["ws_sem"] = din("ws_sem", [BL, 1])
    io["ek_in"] = din("ek_in", [ROWS, KD])
    io["ev_in"] = din("ev_in", [ROWS, KD])
    io["es_in"] = din("es_in", [ROWS, 1])
    io["sk_in"] = din("sk_in", [ROWS, KD])
    io["sv_in"] = din("sv_in", [ROWS, KD])
    io["ss_in"] = din("ss_in", [ROWS, 1])
    io["ptr_epi"] = din("ptr_epi", [BL, 1], I32)
    io["ptr_sem"] = din("ptr_sem", [BL, 1], I32)
    io["Wk"] = din("Wk", [DM, KD])
    io["bk"] = din("bk", [1, KD])
    io["Wv"] = din("Wv", [DM, KD])
    io["bv"] = din("bv", [1, KD])
    io["Wr"] = din("Wr", [KD, DM])
    io["br"] = din("br", [1, DM])

    io["context"] = dout("context", [BL, DM])
    io["all_scores"] = dout("all_scores", [BL, 2 * S])
    io["ek_out"] = dout("ek_out", [ROWS + PAD, KD])
    io["ev_out"] = dout("ev_out", [ROWS + PAD, KD])
    io["es_out"] = dout("es_out", [ROWS + PAD, 1])
    io["sk_out"] = dout("sk_out", [ROWS + PAD, KD])
    io["sv_out"] = dout("sv_out", [ROWS + PAD, KD])
    io["ss_out"] = dout("ss_out", [ROWS + PAD, 1])
    io["ep_out"] = dout("ep_out", [BL, 1], I32)
    io["sp_out"] = dout("sp_out", [BL, 1], I32)
    if DEBUG_TAPS:
        io["dbg_srow"] = dout("dbg_srow", [8, S])
        io["dbg_top32"] = dout("dbg_top32", [8, TOPK])
        io["dbg_idx"] = dout("dbg_idx", [8, TOPK], U32)
        io["dbg_w32"] = dout("dbg_w32", [8, TOPK])
        io["dbg_sel"] = dout("dbg_sel", [TOPK, 8 * KD])
        io["dbg_vals"] = dout("dbg_vals", [KD, BL])

    with tile.TileContext(nc) as tc:
        _emit(tc, io)

    nc.compile()
    return nc


def _emit(tc, io):
    nc = tc.nc
    P = 128
    JPB = S // P  # 64 slot rows per partition in a bank tile

    singles = tc.alloc_tile_pool(name="singles", bufs=1)
    kpool = tc.alloc_tile_pool(name="keys", bufs=2)
    scratch = tc.alloc_tile_pool(name="scratch", bufs=2)
    psum = tc.alloc_tile_pool(name="psum", bufs=2, space="PSUM")
    psum1 = tc.alloc_tile_pool(name="psum1", bufs=1, space="PSUM")

    # ---- constants ----
    identity = singles.tile([P, P], F32)
    make_identity(nc, identity[:])
    ones_r128 = singles.tile([1, P], F32)
    nc.gpsimd.memset(ones_r128[:], 1.0)
    ones_r8 = singles.tile([1, 8], F32)
    nc.gpsimd.memset(ones_r8[:], 1.0)
    inv_chunk = singles.tile([P, 1], F32)
    nc.gpsimd.memset(inv_chunk[:], 1.0 / CHUNK)

    # ---- small loads ----
    wk_sb = singles.tile([P, 8, P], F32)  # [din_in_chunk, chunk, dout]
    nc.sync.dma_start(wk_sb[:], io["Wk"].rearrange("(c p) d -> p c d", p=P))
    wv_sb = singles.tile([P, 8, P], F32)
    nc.sync.dma_start(wv_sb[:], io["Wv"].rearrange("(c p) d -> p c d", p=P))
    wr_sb = singles.tile([P, DM], F32)
    nc.sync.dma_start(wr_sb[:], io["Wr"])
    bk_sb = singles.tile([1, KD], F32)
    nc.sync.dma_start(bk_sb[:], io["bk"])
    bv_sb = singles.tile([1, KD], F32)
    nc.sync.dma_start(bv_sb[:], io["bv"])
    br_sb = singles.tile([1, DM], F32)
    nc.sync.dma_start(br_sb[:], io["br"])
    query_sb = singles.tile([BL, DM], F32)
    nc.sync.dma_start(query_sb[:], io["query"])
    ws_sb = {}
    ptr_sb = {}
    for bank, wsk, ptk in (("E", "ws_epi", "ptr_epi"), ("S", "ws_sem", "ptr_sem")):
        ws_sb[bank] = singles.tile([BL, 1], F32, tag=f"ws{bank}", name=f"ws{bank}")
        nc.sync.dma_start(ws_sb[bank][:], io[wsk])
        ptr_sb[bank] = singles.tile([BL, 1], I32, tag=f"ptr{bank}", name=f"ptr{bank}")
        nc.sync.dma_start(ptr_sb[bank][:], io[ptk])

    # ---- bulk DRAM->DRAM copies: values + score banks (4MiB chunks) ----
    for t_in, t_out in (("ev_in", "ev_out"), ("sv_in", "sv_out")):
        for b in range(BL):
            sl = slice(b * S, (b + 1) * S)
            nc.sync.dma_start(io[t_out][sl, :], io[t_in][sl, :])
    nc.sync.dma_start(io["es_out"][0:ROWS, :], io["es_in"][:])
    nc.sync.dma_start(io["ss_out"][0:ROWS, :], io["ss_in"][:])

    # ---- projections: repT, qT, k_newT, v_newT ----
    # query transpose: 8 chunks [BL,128] -> [128,BL]
    qrT = singles.tile([P, 8, 8], F32)  # [din, chunk, 0:4 queryT | 4:8 repT]
    qrT_ps = psum.tile([P, 32], F32, tag="tmp", name="qrT_ps")
    for dc in range(8):
        nc.tensor.transpose(
            qrT_ps[:, 4 * dc : 4 * dc + 4],
            query_sb[:, dc * P : (dc + 1) * P],
            identity[:BL, :BL],
        )
    nc.vector.tensor_copy(
        qrT[:, :, 0:4], qrT_ps[:].rearrange("p (c b) -> p c b", b=4)
    )
    # rep = mean(hidden, axis=chunk), produced transposed via PE
    repT_ps = psum.tile([P, 8, 4], F32, tag="tmp", name="repT_ps")
    for b in range(BL):
        hb = scratch.tile([P, DM], F32, tag="hidden")
        nc.sync.dma_start(hb[:], io["hidden"][b])
        for dc in range(8):
            nc.tensor.matmul(
                repT_ps[:, dc, b : b + 1],
                hb[:, dc * P : (dc + 1) * P],
                inv_chunk[:],
                start=True,
                stop=True,
            )
    nc.vector.tensor_copy(qrT[:, :, 4:8], repT_ps[:])

    # qkT = Wk^T @ [queryT | repT] + bk  -> cols 0:4 = qT, 4:8 = k_newT
    qkT_ps = psum1.tile([P, 8], F32)
    for dc in range(8):
        nc.tensor.matmul(
            qkT_ps[:],
            wk_sb[:, dc],
            qrT[:, dc],
            start=(dc == 0),
            stop=False,
        )
    nc.tensor.matmul(qkT_ps[:], bk_sb[:], ones_r8[:], start=False, stop=True)
    vT_ps = psum1.tile([P, 4], F32)
    for dc in range(8):
        nc.tensor.matmul(
            vT_ps[:],
            wv_sb[:, dc],
            qrT[:, dc, 4:8],
            start=(dc == 0),
            stop=False,
        )
    nc.tensor.matmul(vT_ps[:], bv_sb[:], ones_r8[:, 0:4], start=False, stop=True)
    qkT = singles.tile([P, 8], F32)
    nc.vector.tensor_copy(qkT[:], qkT_ps[:])
    vT = singles.tile([P, 4], F32)
    nc.vector.tensor_copy(vT[:], vT_ps[:])

    # natural-layout rows: qk_nat [8,128] (0:4 q, 4:8 k_new), v_nat [4,128]
    qk_nat_ps = psum.tile([8, P], F32, tag="tmp", name="qk_nat_ps")
    nc.tensor.transpose(qk_nat_ps[:], qkT[:], identity[:])
    qk_nat = singles.tile([8, P], F32)
    nc.vector.tensor_copy(qk_nat[:], qk_nat_ps[:])
    v_nat_ps = psum.tile([4, P], F32, tag="tmp", name="v_nat_ps")
    nc.tensor.transpose(v_nat_ps[:], vT[:], identity[:])
    v_nat = singles.tile([4, P], F32)
    nc.vector.tensor_copy(v_nat[:], v_nat_ps[:])

    # q broadcast across partitions: q_bcast[:, b*128:(b+1)*128] = q_b
    q_bcast = singles.tile([P, BL, P], F32)
    for b in range(BL):
        qrow_ps = psum.tile([1, P], F32, tag="tmp", name="qrow_ps")
        nc.tensor.transpose(qrow_ps[:], qkT[:, b : b + 1], identity[:])
        qrow = scratch.tile([1, P], F32, tag="qrow_sb")
        nc.vector.tensor_copy(qrow[:], qrow_ps[:])
        qb_ps = psum.tile([P, P], F32, tag="tmp", name="qb_ps")
        nc.tensor.matmul(qb_ps[:], ones_r128[:], qrow[:], start=True, stop=True)
        nc.vector.tensor_copy(q_bcast[:, b], qb_ps[:])

    # ---- keys streaming: scores + copy-out ----
    banks = [("E", "ek_in", "ek_out"), ("S", "sk_in", "sk_out")]
    pairs = [(b, bk_) for bk_ in ("E", "S") for b in range(BL)]  # bb index order
    scores_sb = singles.tile([P, 8, JPB], F32)  # col (bb, j); slot = p*64+j
    scores_row = singles.tile([8, S], F32)  # row bb, natural slot order

    bank_src = {"E": "ek_in", "S": "sk_in"}
    bank_dst = {"E": "ek_out", "S": "sk_out"}
    for bb, (b, bk_) in enumerate(pairs):
        ktile = kpool.tile([P, JPB, P], F32, tag="ktile")
        src = io[bank_src[bk_]][b * S : (b + 1) * S, :]
        nc.sync.dma_start(ktile[:], src.rearrange("(p j) d -> p j d", p=P))
        QJ = 16  # slot rows per DVE pass
        for qr in range(JPB // QJ):
            prod = scratch.tile([P, QJ, P], F32, tag="prod")
            nc.vector.tensor_mul(
                prod[:],
                ktile[:, qr * QJ : (qr + 1) * QJ],
                q_bcast[:, b].unsqueeze(1).to_broadcast([P, QJ, P]),
            )
            nc.vector.tensor_reduce(
                out=scores_sb[:, bb, qr * QJ : (qr + 1) * QJ],
                in_=prod[:],
                axis=mybir.AxisListType.X,
                op=mybir.AluOpType.add,
            )
        dst = io[bank_dst[bk_]][b * S : (b + 1) * S, :]
        nc.sync.dma_start(dst.rearrange("(p j) d -> p j d", p=P), ktile[:])
        # regroup to row layout (before S1 mutates scores_sb)
        nc.sync.dma_start(
            scores_row[bb : bb + 1, :].rearrange("r (p j) -> r p j", p=P),
            scores_sb[:, bb],
        )
        # all_scores output (contiguous per (b,bank))
        col = 0 if bk_ == "E" else S
        nc.sync.dma_start(
            io["all_scores"][b : b + 1, col : col + S].rearrange(
                "r (p j) -> r p j", p=P
            ),
            scores_sb[:, bb],
        )

    # ---- hierarchical exact top-32 ----
    # S1: per-partition top-32 of 64, per bb slice (mutates scores_sb)
    cand1 = singles.tile([P, 8, TOPK], F32)
    for bb in range(8):
        for r in range(4):
            nc.vector.max(cand1[:, bb, 8 * r : 8 * r + 8], scores_sb[:, bb])
            if r < 3:
                nc.vector.match_replace(
                    out=scores_sb[:, bb],
                    in_to_replace=cand1[:, bb, 8 * r : 8 * r + 8],
                    in_values=scores_sb[:, bb],
                    imm_value=NEG,
                )
    # R: collapse per-partition candidates to one row per bb: [8, 4096]
    cand_rows = singles.tile([8, P * TOPK], F32)
    for bb in range(8):
        nc.sync.dma_start(
            cand_rows[bb : bb + 1, :].rearrange("r (p k) -> r p k", p=P),
            cand1[:, bb],
        )
    # S2: top-32 of 4096 per row -> top32 values, descending
    top32 = singles.tile([8, TOPK], F32)
    for r in range(4):
        nc.vector.max(top32[:, 8 * r : 8 * r + 8], cand_rows[:])
        if r < 3:
            nc.vector.match_replace(
                out=cand_rows[:],
                in_to_replace=top32[:, 8 * r : 8 * r + 8],
                in_values=cand_rows[:],
                imm_value=NEG,
            )
    # indices of the top-32 in natural slot order
    idx = singles.tile([8, TOPK], U32)
    for r in range(4):
        nc.vector.max_index(
            idx[:, 8 * r : 8 * r + 8],
            top32[:, 8 * r : 8 * r + 8],
            scores_row[:],
        )

    # ---- softmax over top-32 ----
    neg_m = singles.tile([8, 1], F32)
    nc.vector.tensor_scalar_mul(neg_m[:], top32[:, 0:1], -1.0)
    w32 = singles.tile([8, TOPK], F32)
    sumw = singles.tile([8, 1], F32)
    nc.scalar.activation(
        w32[:],
        top32[:],
        mybir.ActivationFunctionType.Exp,
        bias=neg_m[:],
        accum_out=sumw[:],
    )
    rsum = singles.tile([8, 1], F32)
    nc.vector.reciprocal(rsum[:], sumw[:])
    nc.vector.tensor_scalar_mul(w32[:], w32[:], rsum[:])
    # transpose weights to [32, 8]
    w32T_ps = psum.tile([TOPK, 8], F32, tag="tmp", name="w32T_ps")
    nc.tensor.transpose(w32T_ps[:], w32[:], identity[:8, :8])
    w32T = singles.tile([TOPK, 8], F32)
    nc.vector.tensor_copy(w32T[:], w32T_ps[:])

    # ---- gather top-32 value rows and weighted-sum ----
    idx_f = singles.tile([8, TOPK], F32)
    nc.vector.tensor_copy(idx_f[:], idx[:])
    idxT_ps = psum.tile([TOPK, 8], F32, tag="tmp", name="idxT_ps")
    nc.tensor.transpose(idxT_ps[:], idx_f[:], identity[:8, :8])
    idxT = singles.tile([TOPK, 8], I32)
    nc.vector.tensor_copy(idxT[:], idxT_ps[:])
    off_base = singles.tile([TOPK, 2, 4], I32)
    nc.gpsimd.iota(off_base[:], pattern=[[0, 2], [S, 4]], channel_multiplier=0)
    offs = singles.tile([TOPK, 8], I32)
    nc.vector.tensor_add(offs[:], idxT[:], off_base[:].rearrange("p a b -> p (a b)"))

    sel = singles.tile([TOPK, 8, P], F32)
    bank_vsrc = {"E": "ev_in", "S": "sv_in"}
    for bb, (b, bk_) in enumerate(pairs):
        nc.gpsimd.indirect_dma_start(
            out=sel[:, bb],
            out_offset=None,
            in_=io[bank_vsrc[bk_]][:],
            in_offset=bass.IndirectOffsetOnAxis(ap=offs[:, bb : bb + 1], axis=0),
        )
    vals_ps = psum1.tile([P, 8], F32)
    for bb in range(8):
        nc.tensor.matmul(
            vals_ps[:, bb : bb + 1],
            sel[:, bb],
            w32T[:, bb : bb + 1],
            start=True,
            stop=True,
        )
    if DEBUG_TAPS:
        nc.sync.dma_start(io["dbg_srow"][:], scores_row[:])
        nc.sync.dma_start(io["dbg_top32"][:], top32[:])
        nc.sync.dma_start(io["dbg_idx"][:], idx[:])
        nc.sync.dma_start(io["dbg_w32"][:], w32[:])
        nc.sync.dma_start(io["dbg_sel"][:], sel[:].rearrange("p a b -> p (a b)"))
    vals_sem = singles.tile([P, BL], F32)
    nc.vector.tensor_copy(vals_sem[:], vals_ps[:, 4:8])
    vals = singles.tile([P, BL], F32)
    nc.vector.tensor_add(vals[:], vals_ps[:, 0:4], vals_sem[:])
    if DEBUG_TAPS:
        nc.sync.dma_start(io["dbg_vals"][:], vals[:])

    # ---- context = vals_comb @ Wr + br ----
    for h in range(2):
        ctx_ps = psum.tile([BL, 512], F32, tag="tmp", name="ctx_ps")
        nc.tensor.matmul(
            ctx_ps[:], vals[:], wr_sb[:, h * 512 : (h + 1) * 512], start=True,
            stop=False,
        )
        nc.tensor.matmul(
            ctx_ps[:],
            ones_r8[:, 0:BL],
            br_sb[:, h * 512 : (h + 1) * 512],
            start=False,
            stop=True,
        )
        ctx_sb = scratch.tile([BL, 512], F32, tag="ctx_sb")
        nc.vector.tensor_copy(ctx_sb[:], ctx_ps[:])
        nc.sync.dma_start(io["context"][:, h * 512 : (h + 1) * 512], ctx_sb[:])

    # ---- pointer updates + scatter patches ----
    rowb = singles.tile([BL, 1], I32)
    nc.gpsimd.iota(rowb[:], pattern=[[0, 1]], channel_multiplier=S)
    dump = singles.tile([BL, 1], I32)
    nc.gpsimd.iota(dump[:], pattern=[[0, 1]], base=ROWS, channel_multiplier=1)
    for bk_, pout in (("E", "ep_out"), ("S", "sp_out")):
        mask_f = scratch.tile([BL, 1], F32, tag=f"mf{bk_}")
        nc.vector.tensor_scalar(
            mask_f[:], ws_sb[bk_][:], 0.0, None, op0=mybir.AluOpType.is_gt
        )
        mask_i = scratch.tile([BL, 1], I32, tag=f"mi{bk_}")
        nc.vector.tensor_copy(mask_i[:], mask_f[:])
        pnew = scratch.tile([BL, 1], I32, tag=f"pn{bk_}")
        nc.vector.tensor_add(pnew[:], ptr_sb[bk_][:], mask_i[:])
        nc.sync.dma_start(io[pout][:], pnew[:])
        slot = scratch.tile([BL, 1], I32, tag=f"sl{bk_}")
        nc.vector.tensor_scalar(
            slot[:], ptr_sb[bk_][:], S - 1, None, op0=mybir.AluOpType.bitwise_and
        )
        rows = scratch.tile([BL, 1], I32, tag=f"rw{bk_}")
        nc.vector.tensor_add(rows[:], slot[:], rowb[:])
        rows_f = singles.tile([BL, 1], I32, tag=f"rf{bk_}")
        nc.vector.tensor_copy(rows_f[:], dump[:])
        nc.vector.copy_predicated(rows_f[:], mask_i[:], rows[:])
        kout = "ek_out" if bk_ == "E" else "sk_out"
        vout = "ev_out" if bk_ == "E" else "sv_out"
        sout = "es_out" if bk_ == "E" else "ss_out"
        nc.gpsimd.indirect_dma_start(
            out=io[kout][:],
            out_offset=bass.IndirectOffsetOnAxis(ap=rows_f[:], axis=0),
            in_=qk_nat[4:8, :],
            in_offset=None,
        )
        nc.gpsimd.indirect_dma_start(
            out=io[vout][:],
            out_offset=bass.IndirectOffsetOnAxis(ap=rows_f[:], axis=0),
            in_=v_nat[:],
            in_offset=None,
        )
        nc.gpsimd.indirect_dma_start(
            out=io[sout][:],
            out_offset=bass.IndirectOffsetOnAxis(ap=rows_f[:], axis=0),
            in_=ws_sb[bk_][:],
            in_offset=None,
        )

    for pool in (psum1, psum, scratch, kpool, singles):
        pool.release()


_NC_CACHE = None


def _get_program():
    global _NC_CACHE
    if _NC_CACHE is None:
        _NC_CACHE = _build_program()
    return _NC_CACHE


def kernel(**inputs):
    nc = _get_program()
    hidden = np.asarray(inputs["hidden"], np.float32)
    query = np.asarray(inputs["query"], np.float32)
    ws_e = np.asarray(inputs["write_scores_epi"], np.float32)
    ws_s = np.asarray(inputs["write_scores_sem"], np.float32)
    ek = np.asarray(inputs["episodic_keys"], np.float32)
    ev = np.asarray(inputs["episodic_values"], np.float32)
    es = np.asarray(inputs["episodic_scores"], np.float32)
    sk = np.asarray(inputs["semantic_keys"], np.float32)
    sv = np.asarray(inputs["semantic_values"], np.float32)
    ss = np.asarray(inputs["semantic_scores"], np.float32)
    ep = np.asarray(inputs["episodic_ptr"])
    sp = np.asarray(inputs["semantic_ptr"])
    Wk = np.asarray(inputs["Wk"], np.float32)
    bk = np.asarray(inputs["bk"], np.float32)
    Wv = np.asarray(inputs["Wv"], np.float32)
    bv = np.asarray(inputs["bv"], np.float32)
    Wr = np.asarray(inputs["Wr"], np.float32)
    br = np.asarray(inputs["br"], np.float32)

    in_maps = []
    for c in range(N_CORES):
        sl = slice(c * BL, (c + 1) * BL)
        in_maps.append(
            {
                "hidden": np.ascontiguousarray(hidden[sl]),
                "query": np.ascontiguousarray(query[sl]),
                "ws_epi": np.ascontiguousarray(ws_e[sl]).reshape(BL, 1),
                "ws_sem": np.ascontiguousarray(ws_s[sl]).reshape(BL, 1),
                "ek_in": np.ascontiguousarray(ek[sl]).reshape(ROWS, KD),
                "ev_in": np.ascontiguousarray(ev[sl]).reshape(ROWS, KD),
                "es_in": np.ascontiguousarray(es[sl]).reshape(ROWS, 1),
                "sk_in": np.ascontiguousarray(sk[sl]).reshape(ROWS, KD),
                "sv_in": np.ascontiguousarray(sv[sl]).reshape(ROWS, KD),
                "ss_in": np.ascontiguousarray(ss[sl]).reshape(ROWS, 1),
                "ptr_epi": np.ascontiguousarray(ep[sl]).astype(np.int32).reshape(BL, 1),
                "ptr_sem": np.ascontiguousarray(sp[sl]).astype(np.int32).reshape(BL, 1),
                "Wk": Wk,
                "bk": bk.reshape(1, KD),
                "Wv": Wv,
                "bv": bv.reshape(1, KD),
                "Wr": Wr,
                "br": br.reshape(1, DM),
            }
        )

    res = run_bass_kernel_spmd(nc, in_maps, list(range(N_CORES)))
    outs = res.results

    def gather(name, shape, dt=np.float32, rows=None):
        parts = []
        for c in range(N_CORES):
            a = outs[c][name]
            if rows is not None:
                a = a[:rows]
            parts.append(a.reshape(shape))
        return np.concatenate(parts, axis=0).astype(dt, copy=False)

    context = gather("context", (BL, DM))
    all_scores = gather("all_scores", (BL, 2 * S))
    ek_o = gather("ek_out", (BL, S, KD), rows=ROWS)
    ev_o = gather("ev_out", (BL, S, KD), rows=ROWS)
    es_o = gather("es_out", (BL, S), rows=ROWS)
    sk_o = gather("sk_out", (BL, S, KD), rows=ROWS)
    sv_o = gather("sv_out", (BL, S, KD), rows=ROWS)
    ss_o = gather("ss_out", (BL, S), rows=ROWS)
    ep_o = gather("ep_out", (BL,), dt=np.int32).astype(ep.dtype)
    sp_o = gather("sp_out", (BL,), dt=np.int32).astype(sp.dtype)
    return (context, all_scores, ek_o, ev_o, es_o, ep_o, sk_o, sv_o, ss_o, sp_o)


# revision 18
# speedup vs baseline: 1.2725x; 1.2725x over previous
"""Trainium2 Bass kernel for the BoundedMemory module.

Contract: kernel(**inputs) takes FULL unsharded numpy inputs (as produced by
the reference setup_inputs()) and returns the FULL output tuple:
  (context, all_scores, ek, ev, es, ep, sk, sv, ss, sp)

Sharding: pure data-parallel over batch (B=32 -> 4 per core x 8 cores).

Per-core device program (all shapes hardcoded):
 - keys banks stream through SBUF in whole-bank 4MiB tiles (partition p holds
   64 contiguous slot rows); DVE tensor_tensor_reduce computes q.k dot
   products; the same SBUF tile is DMA'd back out as the updated-bank copy.
 - values banks + score banks are copied DRAM->DRAM (never enter SBUF).
 - exact top-32 via hierarchical vector.max/match_replace reduction, then
   max_index over row-layout scores for the slot indices; top-32 value rows
   are fetched with an indirect gather DMA.
 - circular-buffer scatter writes are done with indirect DMAs whose row
   offsets are computed on device (masked rows redirect to padding rows).
"""

import numpy as np

import concourse.bacc as bacc
import concourse.bass as bass
import concourse.mybir as mybir
import concourse.tile as tile
from concourse.bass_utils import run_bass_kernel_spmd
from concourse.masks import make_identity

B = 32
N_CORES = 8
BL = B // N_CORES  # batch per core = 4
CHUNK = 128
DM = 1024  # d_model
KD = 128  # key_dim == val_dim
S = 8192  # slots per bank
TOPK = 32
ROWS = BL * S  # 32768 rows per bank tensor per core
PAD = 8  # padding rows for masked-write redirect
F32 = mybir.dt.float32
I32 = mybir.dt.int32
U32 = mybir.dt.uint32
NEG = -1.0e30
DEBUG_TAPS = False


def _build_program():
    nc = bacc.Bacc("TRN2", target_bir_lowering=False, debug=False)

    def din(name, shape, dt=F32):
        return nc.dram_tensor(name, shape, dt, kind="ExternalInput").ap()

    def dout(name, shape, dt=F32):
        return nc.dram_tensor(name, shape, dt, kind="ExternalOutput").ap()

    io = {}
    io["hidden"] = din("hidden", [BL, CHUNK, DM])
    io["query"] = din("query", [BL, DM])
    io["ws_epi"] = din("ws_epi", [BL, 1])
    io["ws_sem"] = din("ws_sem", [BL, 1])
    io["ek_in"] = din("ek_in", [ROWS, KD])
    io["ev_in"] = din("ev_in", [ROWS, KD])
    io["es_in"] = din("es_in", [ROWS, 1])
    io["sk_in"] = din("sk_in", [ROWS, KD])
    io["sv_in"] = din("sv_in", [ROWS, KD])
    io["ss_in"] = din("ss_in", [ROWS, 1])
    io["ptr_epi"] = din("ptr_epi", [BL, 1], I32)
    io["ptr_sem"] = din("ptr_sem", [BL, 1], I32)
    io["Wk"] = din("Wk", [DM, KD])
    io["bk"] = din("bk", [1, KD])
    io["Wv"] = din("Wv", [DM, KD])
    io["bv"] = din("bv", [1, KD])
    io["Wr"] = din("Wr", [KD, DM])
    io["br"] = din("br", [1, DM])

    io["context"] = dout("context", [BL, DM])
    io["all_scores"] = dout("all_scores", [BL, 2 * S])
    io["ek_out"] = dout("ek_out", [ROWS + PAD, KD])
    io["ev_out"] = dout("ev_out", [ROWS + PAD, KD])
    io["es_out"] = dout("es_out", [ROWS + PAD, 1])
    io["sk_out"] = dout("sk_out", [ROWS + PAD, KD])
    io["sv_out"] = dout("sv_out", [ROWS + PAD, KD])
    io["ss_out"] = dout("ss_out", [ROWS + PAD, 1])
    io["ep_out"] = dout("ep_out", [BL, 1], I32)
    io["sp_out"] = dout("sp_out", [BL, 1], I32)
    if DEBUG_TAPS:
        io["dbg_srow"] = dout("dbg_srow", [8, S])
        io["dbg_top32"] = dout("dbg_top32", [8, TOPK])
        io["dbg_idx"] = dout("dbg_idx", [8, TOPK], U32)
        io["dbg_w32"] = dout("dbg_w32", [8, TOPK])
        io["dbg_sel"] = dout("dbg_sel", [TOPK, 8 * KD])
        io["dbg_vals"] = dout("dbg_vals", [KD, BL])

    with tile.TileContext(nc) as tc:
        _emit(tc, io)

    nc.compile()
    return nc


def _emit(tc, io):
    nc = tc.nc
    P = 128
    JPB = S // P  # 64 slot rows per partition in a bank tile

    singles = tc.alloc_tile_pool(name="singles", bufs=1)
    kpool = tc.alloc_tile_pool(name="keys", bufs=3)
    scratch = tc.alloc_tile_pool(name="scratch", bufs=2)
    psum = tc.alloc_tile_pool(name="psum", bufs=2, space="PSUM")
    psum1 = tc.alloc_tile_pool(name="psum1", bufs=1, space="PSUM")

    # ---- constants ----
    identity = singles.tile([P, P], F32)
    make_identity(nc, identity[:])
    ones_r128 = singles.tile([1, P], F32)
    nc.gpsimd.memset(ones_r128[:], 1.0)
    ones_r8 = singles.tile([1, 8], F32)
    nc.gpsimd.memset(ones_r8[:], 1.0)
    inv_chunk = singles.tile([P, 1], F32)
    nc.gpsimd.memset(inv_chunk[:], 1.0 / CHUNK)

    # ---- small loads ----
    wk_sb = singles.tile([P, 8, P], F32)  # [din_in_chunk, chunk, dout]
    nc.sync.dma_start(wk_sb[:], io["Wk"].rearrange("(c p) d -> p c d", p=P))
    wv_sb = singles.tile([P, 8, P], F32)
    nc.sync.dma_start(wv_sb[:], io["Wv"].rearrange("(c p) d -> p c d", p=P))
    wr_sb = singles.tile([P, DM], F32)
    nc.sync.dma_start(wr_sb[:], io["Wr"])
    bk_sb = singles.tile([1, KD], F32)
    nc.sync.dma_start(bk_sb[:], io["bk"])
    bv_sb = singles.tile([1, KD], F32)
    nc.sync.dma_start(bv_sb[:], io["bv"])
    br_sb = singles.tile([1, DM], F32)
    nc.sync.dma_start(br_sb[:], io["br"])
    query_sb = singles.tile([BL, DM], F32)
    nc.sync.dma_start(query_sb[:], io["query"])
    ws_sb = {}
    ptr_sb = {}
    for bank, wsk, ptk in (("E", "ws_epi", "ptr_epi"), ("S", "ws_sem", "ptr_sem")):
        ws_sb[bank] = singles.tile([BL, 1], F32, tag=f"ws{bank}", name=f"ws{bank}")
        nc.sync.dma_start(ws_sb[bank][:], io[wsk])
        ptr_sb[bank] = singles.tile([BL, 1], I32, tag=f"ptr{bank}", name=f"ptr{bank}")
        nc.sync.dma_start(ptr_sb[bank][:], io[ptk])

    # ---- projections: repT, qT, k_newT, v_newT ----
    # query transpose: 8 chunks [BL,128] -> [128,BL]
    qrT = singles.tile([P, 8, 8], F32)  # [din, chunk, 0:4 queryT | 4:8 repT]
    qrT_ps = psum.tile([P, 32], F32, tag="tmp", name="qrT_ps")
    for dc in range(8):
        nc.tensor.transpose(
            qrT_ps[:, 4 * dc : 4 * dc + 4],
            query_sb[:, dc * P : (dc + 1) * P],
            identity[:BL, :BL],
        )
    nc.vector.tensor_copy(
        qrT[:, :, 0:4], qrT_ps[:].rearrange("p (c b) -> p c b", b=4)
    )
    # rep = mean(hidden, axis=chunk), produced transposed via PE
    repT_ps = psum.tile([P, 8, 4], F32, tag="tmp", name="repT_ps")
    for b in range(BL):
        hb = scratch.tile([P, DM], F32, tag="hidden")
        nc.sync.dma_start(hb[:], io["hidden"][b])
        for dc in range(8):
            nc.tensor.matmul(
                repT_ps[:, dc, b : b + 1],
                hb[:, dc * P : (dc + 1) * P],
                inv_chunk[:],
                start=True,
                stop=True,
            )
    nc.vector.tensor_copy(qrT[:, :, 4:8], repT_ps[:])

    # qkT = Wk^T @ [queryT | repT] + bk  -> cols 0:4 = qT, 4:8 = k_newT
    qkT_ps = psum1.tile([P, 8], F32)
    for dc in range(8):
        nc.tensor.matmul(
            qkT_ps[:],
            wk_sb[:, dc],
            qrT[:, dc],
            start=(dc == 0),
            stop=False,
        )
    nc.tensor.matmul(qkT_ps[:], bk_sb[:], ones_r8[:], start=False, stop=True)
    vT_ps = psum1.tile([P, 4], F32)
    for dc in range(8):
        nc.tensor.matmul(
            vT_ps[:],
            wv_sb[:, dc],
            qrT[:, dc, 4:8],
            start=(dc == 0),
            stop=False,
        )
    nc.tensor.matmul(vT_ps[:], bv_sb[:], ones_r8[:, 0:4], start=False, stop=True)
    qkT = singles.tile([P, 8], F32)
    nc.vector.tensor_copy(qkT[:], qkT_ps[:])
    vT = singles.tile([P, 4], F32)
    nc.vector.tensor_copy(vT[:], vT_ps[:])

    # natural-layout rows: qk_nat [8,128] (0:4 q, 4:8 k_new), v_nat [4,128]
    qk_nat_ps = psum.tile([8, P], F32, tag="tmp", name="qk_nat_ps")
    nc.tensor.transpose(qk_nat_ps[:], qkT[:], identity[:])
    qk_nat = singles.tile([8, P], F32)
    nc.vector.tensor_copy(qk_nat[:], qk_nat_ps[:])
    v_nat_ps = psum.tile([4, P], F32, tag="tmp", name="v_nat_ps")
    nc.tensor.transpose(v_nat_ps[:], vT[:], identity[:])
    v_nat = singles.tile([4, P], F32)
    nc.vector.tensor_copy(v_nat[:], v_nat_ps[:])

    # q broadcast across partitions: q_bcast[:, b*128:(b+1)*128] = q_b
    q_bcast = singles.tile([P, BL, P], F32)
    for b in range(BL):
        qrow_ps = psum.tile([1, P], F32, tag="tmp", name="qrow_ps")
        nc.tensor.transpose(qrow_ps[:], qkT[:, b : b + 1], identity[:])
        qrow = scratch.tile([1, P], F32, tag="qrow_sb")
        nc.vector.tensor_copy(qrow[:], qrow_ps[:])
        qb_ps = psum.tile([P, P], F32, tag="tmp", name="qb_ps")
        nc.tensor.matmul(qb_ps[:], ones_r128[:], qrow[:], start=True, stop=True)
        nc.vector.tensor_copy(q_bcast[:, b], qb_ps[:])

    # ---- keys streaming: scores + copy-out ----
    banks = [("E", "ek_in", "ek_out"), ("S", "sk_in", "sk_out")]
    pairs = [(b, bk_) for bk_ in ("E", "S") for b in range(BL)]  # bb index order
    scores_sb = singles.tile([P, 8, JPB], F32)  # col (bb, j); slot = p*64+j
    scores_row = singles.tile([8, S], F32)  # row bb, natural slot order
    cand1 = singles.tile([P, 8, TOPK], F32)  # S1 output per bb
    cand_rows = singles.tile([8, P * TOPK], F32)  # candidates, one row per bb

    bank_src = {"E": "ek_in", "S": "sk_in"}
    bank_dst = {"E": "ek_out", "S": "sk_out"}
    for bb, (b, bk_) in enumerate(pairs):
        ktile = kpool.tile([P, JPB, P], F32, tag="ktile")
        src = io[bank_src[bk_]][b * S : (b + 1) * S, :]
        nc.sync.dma_start(ktile[:], src.rearrange("(p j) d -> p j d", p=P))
        QJ = 16  # slot rows per DVE pass
        for qr in range(JPB // QJ):
            prod = scratch.tile([P, QJ, P], F32, tag="prod")
            nc.vector.tensor_mul(
                prod[:],
                ktile[:, qr * QJ : (qr + 1) * QJ],
                q_bcast[:, b].unsqueeze(1).to_broadcast([P, QJ, P]),
            )
            nc.vector.tensor_reduce(
                out=scores_sb[:, bb, qr * QJ : (qr + 1) * QJ],
                in_=prod[:],
                axis=mybir.AxisListType.X,
                op=mybir.AluOpType.add,
            )
        dst = io[bank_dst[bk_]][b * S : (b + 1) * S, :]
        nc.scalar.dma_start(dst.rearrange("(p j) d -> p j d", p=P), ktile[:])
        # regroup to row layout (before S1 mutates scores_sb)
        nc.sync.dma_start(
            scores_row[bb : bb + 1, :].rearrange("r (p j) -> r p j", p=P),
            scores_sb[:, bb],
        )
        # all_scores output (contiguous per (b,bank))
        col = 0 if bk_ == "E" else S
        nc.sync.dma_start(
            io["all_scores"][b : b + 1, col : col + S].rearrange(
                "r (p j) -> r p j", p=P
            ),
            scores_sb[:, bb],
        )
        # S1: per-partition top-32 of 64 (mutates scores_sb after the two
        # DMAs above have read it), collapse to one candidate row per bb
        for r in range(4):
            nc.vector.max(cand1[:, bb, 8 * r : 8 * r + 8], scores_sb[:, bb])
            if r < 3:
                nc.vector.match_replace(
                    out=scores_sb[:, bb],
                    in_to_replace=cand1[:, bb, 8 * r : 8 * r + 8],
                    in_values=scores_sb[:, bb],
                    imm_value=NEG,
                )
        nc.sync.dma_start(
            cand_rows[bb : bb + 1, :].rearrange("r (p k) -> r p k", p=P),
            cand1[:, bb],
        )
    # S2: top-32 of 4096 per row -> top32 values, descending
    top32 = singles.tile([8, TOPK], F32)
    for r in range(4):
        nc.vector.max(top32[:, 8 * r : 8 * r + 8], cand_rows[:])
        if r < 3:
            nc.vector.match_replace(
                out=cand_rows[:],
                in_to_replace=top32[:, 8 * r : 8 * r + 8],
                in_values=cand_rows[:],
                imm_value=NEG,
            )
    # indices of the top-32 in natural slot order
    idx = singles.tile([8, TOPK], U32)
    for r in range(4):
        nc.vector.max_index(
            idx[:, 8 * r : 8 * r + 8],
            top32[:, 8 * r : 8 * r + 8],
            scores_row[:],
        )

    # ---- softmax over top-32 ----
    neg_m = singles.tile([8, 1], F32)
    nc.vector.tensor_scalar_mul(neg_m[:], top32[:, 0:1], -1.0)
    w32 = singles.tile([8, TOPK], F32)
    sumw = singles.tile([8, 1], F32)
    nc.scalar.activation(
        w32[:],
        top32[:],
        mybir.ActivationFunctionType.Exp,
        bias=neg_m[:],
        accum_out=sumw[:],
    )
    rsum = singles.tile([8, 1], F32)
    nc.vector.reciprocal(rsum[:], sumw[:])
    nc.vector.tensor_scalar_mul(w32[:], w32[:], rsum[:])
    # transpose weights to [32, 8]
    w32T_ps = psum.tile([TOPK, 8], F32, tag="tmp", name="w32T_ps")
    nc.tensor.transpose(w32T_ps[:], w32[:], identity[:8, :8])
    w32T = singles.tile([TOPK, 8], F32)
    nc.vector.tensor_copy(w32T[:], w32T_ps[:])

    # ---- gather top-32 value rows and weighted-sum ----
    idx_f = singles.tile([8, TOPK], F32)
    nc.vector.tensor_copy(idx_f[:], idx[:])
    idxT_ps = psum.tile([TOPK, 8], F32, tag="tmp", name="idxT_ps")
    nc.tensor.transpose(idxT_ps[:], idx_f[:], identity[:8, :8])
    idxT = singles.tile([TOPK, 8], I32)
    nc.vector.tensor_copy(idxT[:], idxT_ps[:])
    off_base = singles.tile([TOPK, 2, 4], I32)
    nc.gpsimd.iota(off_base[:], pattern=[[0, 2], [S, 4]], channel_multiplier=0)
    offs = singles.tile([TOPK, 8], I32)
    nc.vector.tensor_add(offs[:], idxT[:], off_base[:].rearrange("p a b -> p (a b)"))

    sel = singles.tile([TOPK, 8, P], F32)
    bank_vsrc = {"E": "ev_in", "S": "sv_in"}
    for bb, (b, bk_) in enumerate(pairs):
        nc.gpsimd.indirect_dma_start(
            out=sel[:, bb],
            out_offset=None,
            in_=io[bank_vsrc[bk_]][:],
            in_offset=bass.IndirectOffsetOnAxis(ap=offs[:, bb : bb + 1], axis=0),
        )
    vals_ps = psum1.tile([P, 8], F32)
    for bb in range(8):
        nc.tensor.matmul(
            vals_ps[:, bb : bb + 1],
            sel[:, bb],
            w32T[:, bb : bb + 1],
            start=True,
            stop=True,
        )
    if DEBUG_TAPS:
        nc.sync.dma_start(io["dbg_srow"][:], scores_row[:])
        nc.sync.dma_start(io["dbg_top32"][:], top32[:])
        nc.sync.dma_start(io["dbg_idx"][:], idx[:])
        nc.sync.dma_start(io["dbg_w32"][:], w32[:])
        nc.sync.dma_start(io["dbg_sel"][:], sel[:].rearrange("p a b -> p (a b)"))
    vals_sem = singles.tile([P, BL], F32)
    nc.vector.tensor_copy(vals_sem[:], vals_ps[:, 4:8])
    vals = singles.tile([P, BL], F32)
    nc.vector.tensor_add(vals[:], vals_ps[:, 0:4], vals_sem[:])
    if DEBUG_TAPS:
        nc.sync.dma_start(io["dbg_vals"][:], vals[:])

    # ---- context = vals_comb @ Wr + br ----
    for h in range(2):
        ctx_ps = psum.tile([BL, 512], F32, tag="tmp", name="ctx_ps")
        nc.tensor.matmul(
            ctx_ps[:], vals[:], wr_sb[:, h * 512 : (h + 1) * 512], start=True,
            stop=False,
        )
        nc.tensor.matmul(
            ctx_ps[:],
            ones_r8[:, 0:BL],
            br_sb[:, h * 512 : (h + 1) * 512],
            start=False,
            stop=True,
        )
        ctx_sb = scratch.tile([BL, 512], F32, tag="ctx_sb")
        nc.vector.tensor_copy(ctx_sb[:], ctx_ps[:])
        nc.sync.dma_start(io["context"][:, h * 512 : (h + 1) * 512], ctx_sb[:])

    # ---- bulk DRAM->DRAM copies: values + score banks (emitted last so
    # they fill trailing DMA bandwidth and don't delay the compute path) ----
    d2d_engines = [nc.gpsimd, nc.scalar, nc.sync]
    i = 0
    for t_in, t_out in (("ev_in", "ev_out"), ("sv_in", "sv_out")):
        for b in range(BL):
            sl = slice(b * S, (b + 1) * S)
            d2d_engines[i % 3].dma_start(io[t_out][sl, :], io[t_in][sl, :])
            i += 1
    nc.gpsimd.dma_start(io["es_out"][0:ROWS, :], io["es_in"][:])
    nc.gpsimd.dma_start(io["ss_out"][0:ROWS, :], io["ss_in"][:])

    # ---- pointer updates + scatter patches ----
    rowb = singles.tile([BL, 1], I32)
    nc.gpsimd.iota(rowb[:], pattern=[[0, 1]], channel_multiplier=S)
    dump = singles.tile([BL, 1], I32)
    nc.gpsimd.iota(dump[:], pattern=[[0, 1]], base=ROWS, channel_multiplier=1)
    for bk_, pout in (("E", "ep_out"), ("S", "sp_out")):
        mask_f = scratch.tile([BL, 1], F32, tag=f"mf{bk_}")
        nc.vector.tensor_scalar(
            mask_f[:], ws_sb[bk_][:], 0.0, None, op0=mybir.AluOpType.is_gt
        )
        mask_i = scratch.tile([BL, 1], I32, tag=f"mi{bk_}")
        nc.vector.tensor_copy(mask_i[:], mask_f[:])
        pnew = scratch.tile([BL, 1], I32, tag=f"pn{bk_}")
        nc.vector.tensor_add(pnew[:], ptr_sb[bk_][:], mask_i[:])
        nc.sync.dma_start(io[pout][:], pnew[:])
        slot = scratch.tile([BL, 1], I32, tag=f"sl{bk_}")
        nc.vector.tensor_scalar(
            slot[:], ptr_sb[bk_][:], S - 1, None, op0=mybir.AluOpType.bitwise_and
        )
        rows = scratch.tile([BL, 1], I32, tag=f"rw{bk_}")
        nc.vector.tensor_add(rows[:], slot[:], rowb[:])
        rows_f = singles.tile([BL, 1], I32, tag=f"rf{bk_}")
        nc.vector.tensor_copy(rows_f[:], dump[:])
        nc.vector.copy_predicated(rows_f[:], mask_i[:], rows[:])
        kout = "ek_out" if bk_ == "E" else "sk_out"
        vout = "ev_out" if bk_ == "E" else "sv_out"
        sout = "es_out" if bk_ == "E" else "ss_out"
        nc.gpsimd.indirect_dma_start(
            out=io[kout][:],
            out_offset=bass.IndirectOffsetOnAxis(ap=rows_f[:], axis=0),
            in_=qk_nat[4:8, :],
            in_offset=None,
        )
        nc.gpsimd.indirect_dma_start(
            out=io[vout][:],
            out_offset=bass.IndirectOffsetOnAxis(ap=rows_f[:], axis=0),
            in_=v_nat[:],
            in_offset=None,
        )
        nc.gpsimd.indirect_dma_start(
            out=io[sout][:],
            out_offset=bass.IndirectOffsetOnAxis(ap=rows_f[:], axis=0),
            in_=ws_sb[bk_][:],
            in_offset=None,
        )

    for pool in (psum1, psum, scratch, kpool, singles):
        pool.release()


_NC_CACHE = None


def _get_program():
    global _NC_CACHE
    if _NC_CACHE is None:
        _NC_CACHE = _build_program()
    return _NC_CACHE


def kernel(**inputs):
    nc = _get_program()
    hidden = np.asarray(inputs["hidden"], np.float32)
    query = np.asarray(inputs["query"], np.float32)
    ws_e = np.asarray(inputs["write_scores_epi"], np.float32)
    ws_s = np.asarray(inputs["write_scores_sem"], np.float32)
    ek = np.asarray(inputs["episodic_keys"], np.float32)
    ev = np.asarray(inputs["episodic_values"], np.float32)
    es = np.asarray(inputs["episodic_scores"], np.float32)
    sk = np.asarray(inputs["semantic_keys"], np.float32)
    sv = np.asarray(inputs["semantic_values"], np.float32)
    ss = np.asarray(inputs["semantic_scores"], np.float32)
    ep = np.asarray(inputs["episodic_ptr"])
    sp = np.asarray(inputs["semantic_ptr"])
    Wk = np.asarray(inputs["Wk"], np.float32)
    bk = np.asarray(inputs["bk"], np.float32)
    Wv = np.asarray(inputs["Wv"], np.float32)
    bv = np.asarray(inputs["bv"], np.float32)
    Wr = np.asarray(inputs["Wr"], np.float32)
    br = np.asarray(inputs["br"], np.float32)

    in_maps = []
    for c in range(N_CORES):
        sl = slice(c * BL, (c + 1) * BL)
        in_maps.append(
            {
                "hidden": np.ascontiguousarray(hidden[sl]),
                "query": np.ascontiguousarray(query[sl]),
                "ws_epi": np.ascontiguousarray(ws_e[sl]).reshape(BL, 1),
                "ws_sem": np.ascontiguousarray(ws_s[sl]).reshape(BL, 1),
                "ek_in": np.ascontiguousarray(ek[sl]).reshape(ROWS, KD),
                "ev_in": np.ascontiguousarray(ev[sl]).reshape(ROWS, KD),
                "es_in": np.ascontiguousarray(es[sl]).reshape(ROWS, 1),
                "sk_in": np.ascontiguousarray(sk[sl]).reshape(ROWS, KD),
                "sv_in": np.ascontiguousarray(sv[sl]).reshape(ROWS, KD),
                "ss_in": np.ascontiguousarray(ss[sl]).reshape(ROWS, 1),
                "ptr_epi": np.ascontiguousarray(ep[sl]).astype(np.int32).reshape(BL, 1),
                "ptr_sem": np.ascontiguousarray(sp[sl]).astype(np.int32).reshape(BL, 1),
                "Wk": Wk,
                "bk": bk.reshape(1, KD),
                "Wv": Wv,
                "bv": bv.reshape(1, KD),
                "Wr": Wr,
                "br": br.reshape(1, DM),
            }
        )

    res = run_bass_kernel_spmd(nc, in_maps, list(range(N_CORES)))
    outs = res.results

    def gather(name, shape, dt=np.float32, rows=None):
        parts = []
        for c in range(N_CORES):
            a = outs[c][name]
            if rows is not None:
                a = a[:rows]
            parts.append(a.reshape(shape))
        return np.concatenate(parts, axis=0).astype(dt, copy=False)

    context = gather("context", (BL, DM))
    all_scores = gather("all_scores", (BL, 2 * S))
    ek_o = gather("ek_out", (BL, S, KD), rows=ROWS)
    ev_o = gather("ev_out", (BL, S, KD), rows=ROWS)
    es_o = gather("es_out", (BL, S), rows=ROWS)
    sk_o = gather("sk_out", (BL, S, KD), rows=ROWS)
    sv_o = gather("sv_out", (BL, S, KD), rows=ROWS)
    ss_o = gather("ss_out", (BL, S), rows=ROWS)
    ep_o = gather("ep_out", (BL,), dt=np.int32).astype(ep.dtype)
    sp_o = gather("sp_out", (BL,), dt=np.int32).astype(sp.dtype)
    return (context, all_scores, ek_o, ev_o, es_o, ep_o, sk_o, sv_o, ss_o, sp_o)
